# revision 28
# baseline (speedup 1.0000x reference)
"""Sequence-parallel self-attention kernel for 8 TRN2 NeuronCores.

Reference computation (N=8192, D=256, fp32):
    q = x @ WQ; k = x @ WK; v = x @ WV
    out = softmax(q @ k.T) @ v

Host->device traffic is the wall-clock bottleneck (axon tunnel ~35 MB/s), so
each core receives ONLY its own fp16 shard plus a 1/8 slice of the packed
weights (~0.55 MB/core instead of 17.8 MB/core replicated), and the full x is
reconstructed on-device with AllGathers over NeuronLink (~14 us each):

  per core c (one fused fp16 input array xw_h [1120, 256]):
    rows 0..1023     own x rows (natural layout)
    rows 1024..1119  rows c*96..(c+1)*96 of packed [WQ; WK.T; WV]
  on device:
    AG#1: cast(xs_h)->f32r, gather -> xg  [8192, 256]   (natural x)
    AG#2: XBAR dma-transpose(xs_h)->f32r, gather -> xgT [2048, 1024]
          (8 stacked [256,1024] per-core transposed shards)
    AG#3: gather w_h -> w_all [768, 256] fp16, cast -> f32r weight tiles

Per-core algebra (identical to the proven replicated-input kernel; everything
stays transposed so softmax's k-reduction is a partition-axis ones-matmul):
    qT = WQ.T @ xT_local                      [256, 1024]
    M  = WK @ qT        (lhsT = WK.T)         [256, 1024]
    per k-chunk c (64 chunks of 128):
      scoresT = x_c @ M                       [128, 1024]   (= (q @ k.T).T chunk)
      expT    = exp(scoresT - 15)             (constant shift cancels in softmax)
      sums   += ones[128,1].T @ expT          [1, 1024]     (softmax denominator)
      UT     += x_c.T @ expT                  [256, 1024]   (= (attn_unnorm @ x).T)
    UTn  = UT * broadcast(1/sums)
    outT = WV.T @ UTn                         [256, 1024]   (int8 + f32 absmax
                                                             scale; host
                                                             dequantizes + .T)

All matmuls run as float32r (full PE rate at free-dim >= 256). fp16 input
transport + f32r compute + int8 output quantization land at ~5.7e-3 rel err
-- 3.5x under the 2e-2 gate.

Hardware quirk found while tuning: late SBUF->DRAM stores issued on the sync
DMA queue corrupt their payload in this build (every 32-bit word of some 4KB
spans gets +0x800 added then its low 12 bits cleared -- an fp32-mantissa-style
rounding). The output stores therefore go through the gpsimd DMA queue, which
is unaffected.
"""

import numpy as np

N, D, P = 8192, 256, 8
NL = N // P          # 1024 q-rows per core
KC = 128             # k-chunk size (contraction tile)
NCHUNK = N // KC     # 64
SB = 8               # k-chunks per DMA superblock
WSH = 3 * D // P     # 96 packed-weight rows per core
QLEV = 2047.0        # 12-bit x quantization: x ~ (v - QLEV) * (absmax/QLEV)
WROWS = WSH * 4      # fp16 weight-shard bytes as 384 uint8 rows of 128
XU_ROWS = 3 * NL + WROWS + 1
EXP_SHIFT = -15.0    # exp(s - 15): keeps ACT exp-table args in a good range
QDEN = 120.0         # int8 quant denominator; headroom vs 127 absorbs the
                     # ~1% error of the DVE reciprocal so +max never wraps

_CACHE = {}


def _build():
    import concourse.bacc as bacc
    import concourse.mybir as mybir
    import concourse.tile as tile

    import concourse.bass_isa as bass_isa

    f32 = mybir.dt.float32
    f32r = mybir.dt.float32r
    f16 = mybir.dt.float16
    i8 = mybir.dt.int8
    EXP = mybir.ActivationFunctionType.Exp
    COPY = mybir.ActivationFunctionType.Copy
    RG = [list(range(P))]

    nc = bacc.Bacc("TRN2", target_bir_lowering=False, debug=False,
                   enable_asserts=False, num_devices=P,
                   enable_partition_id=False)

    u8 = mybir.dt.uint8
    AL = mybir.AluOpType
    xu8 = nc.dram_tensor("xu8", [XU_ROWS, 128], u8, kind="ExternalInput").ap()
    outT = nc.dram_tensor("outT", [D + 1, NL], i8, kind="ExternalOutput").ap()

    with tile.TileContext(nc) as tc:
        with (
            tc.tile_pool(name="const", bufs=1) as cpool,
            tc.tile_pool(name="proj", bufs=1) as ppool,
            tc.tile_pool(name="xts", bufs=4) as xtpool,
            tc.tile_pool(name="xns", bufs=4) as xnpool,
            tc.tile_pool(name="expt", bufs=8) as epool,
            tc.tile_pool(name="tail", bufs=1) as tpool,
            tc.tile_pool(name="dram", bufs=1, space="DRAM") as dpool,
            tc.tile_pool(name="ps_scores", bufs=2, space="PSUM") as ps_s,
            tc.tile_pool(name="ps_ut", bufs=1, space="PSUM") as ps_ut,
            tc.tile_pool(name="ps_sums", bufs=1, space="PSUM") as ps_sum,
        ):
            # ---- decode quant scale (f32 bytes in the last packed row) ----
            s_one = cpool.tile([1, 1], f32, tag="s_one", name="s_one")
            nc.sync.dma_start(s_one[:], xu8[3 * NL + WROWS:, 0:4].bitcast(f32))
            s_b = cpool.tile([128, 1], f32, tag="s_b", name="s_b")
            nc.gpsimd.partition_broadcast(s_b[:], s_one[:], channels=128)
            s256_b = cpool.tile([128, 1], f32, tag="s256_b", name="s256_b")
            nc.vector.tensor_scalar_mul(s256_b[:], s_b[:], 256.0)
            sbias_b = cpool.tile([128, 1], f32, tag="sbias_b", name="sbias_b")
            nc.vector.tensor_scalar_mul(sbias_b[:], s_b[:], -QLEV)

            # ---- unpack 12-bit x planes -> f32r natural-layout shard ----
            # v_lo = B0 + 256*(B1 & 0xF); v_hi = B2 + 256*(B1 >> 4)
            # x = (v - QLEV) * s, halves laid out at d-cols [0:128) / [128:256)
            bf = []
            for pl in range(3):
                bu = cpool.tile([128, SB, 128], u8, tag=f"bu{pl}", name=f"bu{pl}")
                nc.sync.dma_start(
                    bu[:],
                    xu8[pl * NL:(pl + 1) * NL, :].rearrange("(a p) d -> p a d", p=128))
                f = cpool.tile([128, SB, 128], f32, tag=f"bf{pl}", name=f"bf{pl}")
                nc.vector.tensor_copy(f[:], bu[:])
                bf.append(f)
            nib = []
            for nm, sh_op, operand in (("lo", AL.bitwise_and, 15),
                                       ("hi", AL.logical_shift_right, 4)):
                t32 = cpool.tile([128, SB, 128], mybir.dt.int32, tag=f"n32{nm}",
                                 name=f"n32{nm}")
                nc.vector.tensor_copy(t32[:], bf[1][:])
                tn = cpool.tile([128, SB, 128], mybir.dt.int32, tag=f"ni{nm}",
                                name=f"ni{nm}")
                nc.vector.tensor_scalar(tn[:], t32[:], operand, None, op0=sh_op)
                tf = cpool.tile([128, SB, 128], f32, tag=f"nf{nm}", name=f"nf{nm}")
                nc.vector.tensor_copy(tf[:], tn[:])
                nib.append(tf)
            xs_sb = cpool.tile([128, SB, D], f32r, tag="xs_sb", name="xs_sb")
            tmp = cpool.tile([128, SB, 128], f32, tag="unp_tmp", name="unp_tmp")
            with nc.allow_low_precision(reason="f32r same mantissa path"):
                for half, (base, nf) in enumerate(((bf[0], nib[0]),
                                                   (bf[2], nib[1]))):
                    nc.vector.tensor_scalar(
                        tmp[:], nf[:], s256_b[:], sbias_b[:],
                        op0=AL.mult, op1=AL.add)
                    nc.vector.scalar_tensor_tensor(
                        xs_sb[:, :, half * 128:(half + 1) * 128], base[:],
                        s_b[:], tmp[:], op0=AL.mult, op1=AL.add)
            xs_int = dpool.tile([NL, D], f32r, tag="xs_int", name="xs_int")
            nc.sync.dma_start(
                xs_int[:].rearrange("(a p) d -> p a d", p=128), xs_sb[:])
            xg = dpool.tile([N, D], f32r, tag="xg", name="xg",
                            addr_space="Shared")
            nc.gpsimd.collective_compute(
                "AllGather", mybir.AluOpType.bypass, replica_groups=RG,
                ins=[xs_int.opt()], outs=[xg.opt()])

            # ---- transposed shard via PE transpose (identity matmul) ----
            id_f32 = cpool.tile([128, 128], f32, tag="id_f32", name="id_f32")
            nc.vector.memset(id_f32[:], 1.0)
            id_sel = cpool.tile([128, 128], f32, tag="id_sel", name="id_sel")
            nc.gpsimd.affine_select(
                id_sel[:], id_f32[:], pattern=[[-1, 128]],
                compare_op=AL.is_equal, fill=0.0, base=0,
                channel_multiplier=1)
            ident = cpool.tile([128, 128], f32r, tag="ident", name="ident")
            nc.vector.tensor_copy(ident[:], id_sel[:])
            xsT_sb = [cpool.tile([128, NL], f32r, tag=f"trf{h}", name=f"trf{h}")
                      for h in range(2)]
            xsT_int = dpool.tile([D, NL], f32r, tag="xsT_int", name="xsT_int")
            for dh in range(2):
                for j in range(SB):
                    pp = ps_s.tile([128, 512], f32, tag="scores", name="scores")
                    nc.tensor.matmul(
                        pp[:, 0:128], xs_sb[:, j, dh * 128:(dh + 1) * 128],
                        ident[:], start=True, stop=True)
                    nc.vector.tensor_copy(
                        xsT_sb[dh][:, j * 128:(j + 1) * 128], pp[:, 0:128])
                nc.sync.dma_start(xsT_int[dh * 128:(dh + 1) * 128, :],
                                  xsT_sb[dh][:])
            xgT = dpool.tile([P * D, NL], f32r, tag="xgT", name="xgT",
                             addr_space="Shared")
            nc.gpsimd.collective_compute(
                "AllGather", mybir.AluOpType.bypass, replica_groups=RG,
                ins=[xsT_int.opt()], outs=[xgT.opt()])

            # packed weights [WQ; WK.T; WV]: fp16 bytes inside xu8 -> AG
            w_sb_h = cpool.tile([WSH, D], f16, tag="w_sb_h", name="w_sb_h")
            nc.sync.dma_start(
                w_sb_h[:],
                xu8[3 * NL:3 * NL + WROWS, :].bitcast(f16)
                .rearrange("(a b) c -> a (b c)", b=4))
            w_int = dpool.tile([WSH, D], f16, tag="w_int", name="w_int")
            nc.sync.dma_start(w_int[:], w_sb_h[:])
            w_all = dpool.tile([3 * D, D], f16, tag="w_all", name="w_all",
                               addr_space="Shared")
            nc.gpsimd.collective_compute(
                "AllGather", mybir.AluOpType.bypass, replica_groups=RG,
                ins=[w_int.opt()], outs=[w_all.opt()])

            def wtiles(base, nm):
                out = []
                for h in range(2):
                    th = cpool.tile([128, D], f16, tag=f"{nm}h{h}",
                                    name=f"{nm}h{h}")
                    nc.sync.dma_start(
                        th[:], w_all[base + h * 128: base + (h + 1) * 128, :])
                    tf = cpool.tile([128, D], f32r, tag=f"{nm}{h}",
                                    name=f"{nm}{h}")
                    nc.vector.tensor_copy(tf[:], th[:])
                    out.append(tf)
                return out

            wq_t = wtiles(0, "wq")
            wkt_t = wtiles(D, "wkt")
            wv_t = wtiles(2 * D, "wv")

            # ---- constants ----
            ones_col = cpool.tile([128, 1], f32r, tag="ones_col", name="ones_col")
            ones_row = cpool.tile([1, 128], f32r, tag="ones_row", name="ones_row")
            ones_f32 = cpool.tile([128, 1], f32, tag="ones_f32", name="ones_f32")
            ones_f32r = cpool.tile([1, 128], f32, tag="ones_f32r", name="ones_f32r")
            bias_t = cpool.tile([128, 1], f32, tag="bias_t", name="bias_t")
            nc.vector.memset(ones_f32[:], 1.0)
            nc.vector.memset(ones_f32r[:], 1.0)
            nc.vector.tensor_copy(ones_col[:], ones_f32[:])
            nc.vector.tensor_copy(ones_row[:], ones_f32r[:])
            nc.vector.memset(bias_t[:], EXP_SHIFT)

            # ---- qT = WQ.T @ xT_local ; M = WK @ qT ----
            qT_t = [ppool.tile([128, NL], f32r, tag=f"qt{h}", name=f"qt{h}") for h in range(2)]
            m_t = [ppool.tile([128, NL], f32r, tag=f"m{h}", name=f"m{h}") for h in range(2)]
            for dst, lhs in ((qT_t, wq_t), (m_t, wkt_t)):
                src = xsT_sb if dst is qT_t else qT_t
                for mh in range(2):
                    for nh in range(2):
                        pp = ps_s.tile([128, 512], f32, tag="scores", name="scores")
                        for kp in range(2):
                            nc.tensor.matmul(
                                pp[:],
                                lhs[kp][:, mh * 128:(mh + 1) * 128],
                                src[kp][:, nh * 512:(nh + 1) * 512],
                                start=(kp == 0), stop=(kp == 1),
                            )
                        nc.vector.tensor_copy(
                            dst[mh][:, nh * 512:(nh + 1) * 512], pp[:])

            # ---- persistent accumulators ----
            ut_ps = [ps_ut.tile([128, NL], f32, tag=f"ut{h}", name=f"ut{h}") for h in range(2)]
            sums_ps = [ps_sum.tile([1, 512], f32, tag=f"sums{h}", name=f"sums{h}")
                       for h in range(2)]

            # ---- main k-loop ----
            for sb in range(N // (KC * SB)):
                xt_t = [xtpool.tile([128, KC * SB], f32r, tag=f"xt{h}", name=f"xt{h}")
                        for h in range(2)]
                for h in range(2):
                    nc.sync.dma_start(
                        xt_t[h][:],
                        xgT[sb * 2 * 128 + h * 128:sb * 2 * 128 + (h + 1) * 128,
                            :])
                xn_t = xnpool.tile([128, SB, D], f32r, tag="xn", name="xn")
                nc.sync.dma_start(
                    xn_t[:],
                    xg[sb * KC * SB:(sb + 1) * KC * SB, :]
                    .rearrange("(a p) d -> p a d", p=128))

                for j in range(SB):
                    c = sb * SB + j
                    first, last = (c == 0), (c == NCHUNK - 1)
                    exps = []
                    for qh in range(2):
                        sp = ps_s.tile([128, 512], f32, tag="scores", name="scores")
                        for kp in range(2):
                            nc.tensor.matmul(
                                sp[:],
                                xt_t[kp][:, j * KC:(j + 1) * KC],
                                m_t[kp][:, qh * 512:(qh + 1) * 512],
                                start=(kp == 0), stop=(kp == 1),
                            )
                        et = epool.tile([128, 512], f32r, tag="expt", name="expt")
                        nc.scalar.activation(et[:], sp[:], EXP, bias=bias_t[:])
                        exps.append(et)
                    for qh in range(2):
                        et = exps[qh]
                        nc.tensor.matmul(
                            sums_ps[qh][:], ones_col[:], et[:],
                            start=first, stop=last)
                        for dh in range(2):
                            nc.tensor.matmul(
                                ut_ps[dh][:, qh * 512:(qh + 1) * 512],
                                xn_t[:, j, dh * 128:(dh + 1) * 128],
                                et[:],
                                start=first, stop=last)

            # ---- tail: softmax normalize + WV projection ----
            sums_sb = tpool.tile([1, NL], f32, tag="sums_sb", name="sums_sb")
            for qh in range(2):
                nc.vector.tensor_copy(
                    sums_sb[:, qh * 512:(qh + 1) * 512], sums_ps[qh][:])
            recip_sb = tpool.tile([1, NL], f32r, tag="recip_sb", name="recip_sb")
            with nc.allow_low_precision(reason="f32r is 4-byte, same mantissa path"):
                nc.vector.reciprocal(recip_sb[:], sums_sb[:])

            rb_sb = tpool.tile([128, NL], f32, tag="rb_sb", name="rb_sb")
            for qh in range(2):
                rp = ps_s.tile([128, 512], f32, tag="scores", name="scores")
                nc.tensor.matmul(
                    rp[:], ones_row[:],
                    recip_sb[:, qh * 512:(qh + 1) * 512],
                    start=True, stop=True)
                nc.vector.tensor_copy(rb_sb[:, qh * 512:(qh + 1) * 512], rp[:])

            utn_sb = [tpool.tile([128, NL], f32r, tag=f"utn{h}", name=f"utn{h}")
                      for h in range(2)]
            for dh in range(2):
                nc.vector.tensor_mul(utn_sb[dh][:], ut_ps[dh][:], rb_sb[:])

            # WV projection into two live PSUM tiles, then int8-quantize with a
            # per-core absmax scale (int8 + f32 scale halves the output bytes;
            # quantization error ~m/240 is ~4e-3 of the rel-err denominator)
            o_f32 = []
            am = tpool.tile([128, 2], f32, tag="am", name="am")
            for mh in range(2):
                op = ps_ut.tile([128, NL], f32, tag=f"ut{mh}", name=f"ut{mh}")
                for nh in range(2):
                    for kp in range(2):
                        nc.tensor.matmul(
                            op[:, nh * 512:(nh + 1) * 512],
                            wv_t[kp][:, mh * 128:(mh + 1) * 128],
                            utn_sb[kp][:, nh * 512:(nh + 1) * 512],
                            start=(kp == 0), stop=(kp == 1),
                        )
                of = tpool.tile([128, NL], f32, tag=f"of{mh}", name=f"of{mh}")
                nc.vector.tensor_copy(of[:], op[:])
                nc.vector.reduce_max(
                    am[:, mh:mh + 1], of[:], axis=mybir.AxisListType.X,
                    apply_absolute_value=True)
                o_f32.append(of)
            amax = tpool.tile([128, 1], f32, tag="amax", name="amax")
            nc.vector.reduce_max(amax[:], am[:], axis=mybir.AxisListType.X)
            nc.gpsimd.partition_all_reduce(
                amax[:], amax[:], channels=128,
                reduce_op=bass_isa.ReduceOp.absmax)
            sc126 = tpool.tile([128, 1], f32, tag="sc126", name="sc126")
            nc.scalar.activation(sc126[:], amax[:], COPY, scale=1.0 / QDEN)
            rcp = tpool.tile([128, 1], f32, tag="rcp", name="rcp")
            nc.vector.reciprocal(rcp[:], sc126[:])
            o_sb = [tpool.tile([128, NL], i8, tag=f"osb{h}", name=f"osb{h}") for h in range(2)]
            for mh in range(2):
                with nc.allow_low_precision(reason="int8 output transport"):
                    nc.vector.tensor_scalar_mul(o_sb[mh][:], o_f32[mh][:],
                                                rcp[:])
                # gpsimd queue, NOT sync: late sync-queue stores corrupt the
                # payload in this build (32-bit words get an fp32-style
                # low-12-bit rounding); the gpsimd DGE ring is clean.
                nc.gpsimd.dma_start(
                    outT[mh * 128:(mh + 1) * 128, :], o_sb[mh][:])
            # absmax f32 bitcast to 4 bytes, packed into outT's extra row
            nc.gpsimd.dma_start(outT[D:D + 1, 0:4],
                                amax[0:1, 0:1].bitcast(i8))

    nc.compile()
    return nc


def _setup_jax_cache():
    """Persistent XLA compilation cache: run_bass_kernel_spmd re-jits a fresh
    closure every call, so without this each call pays ~100ms of XLA
    recompile for the identical HLO."""
    if "jaxcache" in _CACHE:
        return
    import jax

    jax.config.update("jax_compilation_cache_dir", "/tmp/jaxcache")
    jax.config.update("jax_persistent_cache_min_entry_size_bytes", 0)
    jax.config.update("jax_persistent_cache_min_compile_time_secs", 0)
    _CACHE["jaxcache"] = True


def _get_nc():
    if "nc" not in _CACHE:
        _setup_jax_cache()
        _CACHE["nc"] = _build()
    return _CACHE["nc"]


def make_in_maps(input, WQ, WK, WV):
    """Per-core input: one uint8 array packing 12-bit-quantized x (3 byte
    planes), the fp16 weight shard bytes, and the f32 quant scale."""
    x = np.ascontiguousarray(input, dtype=np.float32)
    s = float(np.abs(x).max()) / QLEV
    xq = (np.clip(np.round(x / s), -QLEV, QLEV) + QLEV).astype(np.int32)
    wpack = np.concatenate(
        [np.asarray(WQ, dtype=np.float32),
         np.asarray(WK, dtype=np.float32).T,
         np.asarray(WV, dtype=np.float32)], axis=0).astype(np.float16)
    srow = np.zeros((1, 128), np.uint8)
    srow[0, 0:4] = np.frombuffer(np.float32(s).tobytes(), np.uint8)
    maps = []
    for c in range(P):
        v0 = xq[c * NL:(c + 1) * NL, 0:128]
        v1 = xq[c * NL:(c + 1) * NL, 128:256]
        b0 = (v0 & 0xFF).astype(np.uint8)
        b2 = (v1 & 0xFF).astype(np.uint8)
        b1 = ((v0 >> 8) | ((v1 >> 8) << 4)).astype(np.uint8)
        wb = wpack[c * WSH:(c + 1) * WSH].view(np.uint8).reshape(WROWS, 128)
        maps.append({"xu8": np.ascontiguousarray(
            np.concatenate([b0, b1, b2, wb, srow], axis=0))})
    return maps


def kernel(input, WQ, WK, WV):
    from concourse import bass_utils

    nc = _get_nc()
    in_maps = make_in_maps(input, WQ, WK, WV)
    res = bass_utils.run_bass_kernel_spmd(nc, in_maps, core_ids=list(range(P)))
    out = np.empty((N, D), dtype=np.float32)
    for c in range(P):
        o = res.results[c]["outT"]
        amax = np.frombuffer(o[D, 0:4].tobytes(), np.float32)[0]
        out[c * NL:(c + 1) * NL, :] = (
            o[:D].astype(np.float32) * (float(amax) / QDEN)).T
    return out


# revision 29
# speedup vs baseline: 1.0397x; 1.0397x over previous
"""Sequence-parallel self-attention kernel for 8 TRN2 NeuronCores.

Reference computation (N=8192, D=256, fp32):
    q = x @ WQ; k = x @ WK; v = x @ WV
    out = softmax(q @ k.T) @ v

Host->device traffic is the wall-clock bottleneck (axon tunnel ~35 MB/s), so
each core receives ONLY its own fp16 shard plus a 1/8 slice of the packed
weights (~0.55 MB/core instead of 17.8 MB/core replicated), and the full x is
reconstructed on-device with AllGathers over NeuronLink (~14 us each):

  per core c (one fused fp16 input array xw_h [1120, 256]):
    rows 0..1023     own x rows (natural layout)
    rows 1024..1119  rows c*96..(c+1)*96 of packed [WQ; WK.T; WV]
  on device:
    AG#1: cast(xs_h)->f32r, gather -> xg  [8192, 256]   (natural x)
    AG#2: XBAR dma-transpose(xs_h)->f32r, gather -> xgT [2048, 1024]
          (8 stacked [256,1024] per-core transposed shards)
    AG#3: gather w_h -> w_all [768, 256] fp16, cast -> f32r weight tiles

Per-core algebra (identical to the proven replicated-input kernel; everything
stays transposed so softmax's k-reduction is a partition-axis ones-matmul):
    qT = WQ.T @ xT_local                      [256, 1024]
    M  = WK @ qT        (lhsT = WK.T)         [256, 1024]
    per k-chunk c (64 chunks of 128):
      scoresT = x_c @ M                       [128, 1024]   (= (q @ k.T).T chunk)
      expT    = exp(scoresT - 15)             (constant shift cancels in softmax)
      sums   += ones[128,1].T @ expT          [1, 1024]     (softmax denominator)
      UT     += x_c.T @ expT                  [256, 1024]   (= (attn_unnorm @ x).T)
    UTn  = UT * broadcast(1/sums)
    outT = WV.T @ UTn                         [256, 1024]   (int8 + f32 absmax
                                                             scale; host
                                                             dequantizes + .T)

All matmuls run as float32r (full PE rate at free-dim >= 256). fp16 input
transport + f32r compute + int8 output quantization land at ~5.7e-3 rel err
-- 3.5x under the 2e-2 gate.

Hardware quirk found while tuning: late SBUF->DRAM stores issued on the sync
DMA queue corrupt their payload in this build (every 32-bit word of some 4KB
spans gets +0x800 added then its low 12 bits cleared -- an fp32-mantissa-style
rounding). The output stores therefore go through the gpsimd DMA queue, which
is unaffected.
"""

import numpy as np

N, D, P = 8192, 256, 8
NL = N // P          # 1024 q-rows per core
KC = 128             # k-chunk size (contraction tile)
NCHUNK = N // KC     # 64
SB = 8               # k-chunks per DMA superblock
WSH = 3 * D // P     # 96 packed-weight rows per core
EXP_SHIFT = -15.0    # exp(s - 15): keeps ACT exp-table args in a good range
QDEN = 120.0         # int8 quant denominator; headroom vs 127 absorbs the
                     # ~1% error of the DVE reciprocal so +max never wraps

_CACHE = {}


def _build():
    import concourse.bacc as bacc
    import concourse.mybir as mybir
    import concourse.tile as tile

    import concourse.bass_isa as bass_isa

    f32 = mybir.dt.float32
    f32r = mybir.dt.float32r
    f16 = mybir.dt.float16
    i8 = mybir.dt.int8
    EXP = mybir.ActivationFunctionType.Exp
    COPY = mybir.ActivationFunctionType.Copy
    RG = [list(range(P))]

    nc = bacc.Bacc("TRN2", target_bir_lowering=False, debug=False,
                   enable_asserts=False, num_devices=P,
                   enable_partition_id=False)

    xw_h = nc.dram_tensor("xw_h", [NL + WSH, D], f16, kind="ExternalInput").ap()
    xs_h = xw_h[0:NL, :]
    w_hs = xw_h[NL:NL + WSH, :]
    outT = nc.dram_tensor("outT", [D + 1, NL], i8, kind="ExternalOutput").ap()

    with tile.TileContext(nc) as tc:
        with (
            tc.tile_pool(name="const", bufs=1) as cpool,
            tc.tile_pool(name="proj", bufs=1) as ppool,
            tc.tile_pool(name="xts", bufs=4) as xtpool,
            tc.tile_pool(name="xns", bufs=4) as xnpool,
            tc.tile_pool(name="expt", bufs=8) as epool,
            tc.tile_pool(name="tail", bufs=1) as tpool,
            tc.tile_pool(name="dram", bufs=1, space="DRAM") as dpool,
            tc.tile_pool(name="ps_scores", bufs=2, space="PSUM") as ps_s,
            tc.tile_pool(name="ps_ut", bufs=1, space="PSUM") as ps_ut,
            tc.tile_pool(name="ps_sums", bufs=1, space="PSUM") as ps_sum,
        ):
            # ---- stage own shard + weights into DRAM, AllGather ----
            # natural-layout shard: fp16 -> SBUF -> f32r -> local DRAM -> AG
            xs_sb_h = cpool.tile([128, SB, D], f16, tag="xs_sb_h", name="xs_sb_h")
            nc.sync.dma_start(
                xs_sb_h[:], xs_h[:].rearrange("(a p) d -> p a d", p=128))
            xs_sb = cpool.tile([128, SB, D], f32r, tag="xs_sb", name="xs_sb")
            nc.vector.tensor_copy(xs_sb[:], xs_sb_h[:])
            xs_int = dpool.tile([NL, D], f32r, tag="xs_int", name="xs_int")
            nc.sync.dma_start(
                xs_int[:].rearrange("(a p) d -> p a d", p=128), xs_sb[:])
            xg = dpool.tile([N, D], f32r, tag="xg", name="xg",
                            addr_space="Shared")
            nc.gpsimd.collective_compute(
                "AllGather", mybir.AluOpType.bypass, replica_groups=RG,
                ins=[xs_int.opt()], outs=[xg.opt()])

            # transposed shard via XBAR dma-transpose: fp16 -> f32r -> AG
            xsT_sb = []
            xsT_int = dpool.tile([D, NL], f32r, tag="xsT_int", name="xsT_int")
            for h in range(2):
                trh = cpool.tile([128, NL], f16, tag=f"trh{h}", name=f"trh{h}")
                nc.sync.dma_start(
                    trh[:], xs_h[:, h * 128:(h + 1) * 128], transpose=True)
                trf = cpool.tile([128, NL], f32r, tag=f"trf{h}", name=f"trf{h}")
                nc.vector.tensor_copy(trf[:], trh[:])
                nc.sync.dma_start(xsT_int[h * 128:(h + 1) * 128, :], trf[:])
                xsT_sb.append(trf)
            xgT = dpool.tile([P * D, NL], f32r, tag="xgT", name="xgT",
                             addr_space="Shared")
            nc.gpsimd.collective_compute(
                "AllGather", mybir.AluOpType.bypass, replica_groups=RG,
                ins=[xsT_int.opt()], outs=[xgT.opt()])

            # packed weights [WQ; WK.T; WV]: shard -> AG -> SBUF f32r tiles
            w_sb_h = cpool.tile([WSH, D], f16, tag="w_sb_h", name="w_sb_h")
            nc.sync.dma_start(w_sb_h[:], w_hs)
            w_int = dpool.tile([WSH, D], f16, tag="w_int", name="w_int")
            nc.sync.dma_start(w_int[:], w_sb_h[:])
            w_all = dpool.tile([3 * D, D], f16, tag="w_all", name="w_all",
                               addr_space="Shared")
            nc.gpsimd.collective_compute(
                "AllGather", mybir.AluOpType.bypass, replica_groups=RG,
                ins=[w_int.opt()], outs=[w_all.opt()])

            def wtiles(base, nm):
                out = []
                for h in range(2):
                    th = cpool.tile([128, D], f16, tag=f"{nm}h{h}",
                                    name=f"{nm}h{h}")
                    nc.sync.dma_start(
                        th[:], w_all[base + h * 128: base + (h + 1) * 128, :])
                    tf = cpool.tile([128, D], f32r, tag=f"{nm}{h}",
                                    name=f"{nm}{h}")
                    nc.vector.tensor_copy(tf[:], th[:])
                    out.append(tf)
                return out

            wq_t = wtiles(0, "wq")
            wkt_t = wtiles(D, "wkt")
            wv_t = wtiles(2 * D, "wv")

            # ---- constants ----
            ones_col = cpool.tile([128, 1], f32r, tag="ones_col", name="ones_col")
            ones_row = cpool.tile([1, 128], f32r, tag="ones_row", name="ones_row")
            ones_f32 = cpool.tile([128, 1], f32, tag="ones_f32", name="ones_f32")
            ones_f32r = cpool.tile([1, 128], f32, tag="ones_f32r", name="ones_f32r")
            bias_t = cpool.tile([128, 1], f32, tag="bias_t", name="bias_t")
            nc.vector.memset(ones_f32[:], 1.0)
            nc.vector.memset(ones_f32r[:], 1.0)
            nc.vector.tensor_copy(ones_col[:], ones_f32[:])
            nc.vector.tensor_copy(ones_row[:], ones_f32r[:])
            nc.vector.memset(bias_t[:], EXP_SHIFT)

            # ---- qT = WQ.T @ xT_local ; M = WK @ qT ----
            qT_t = [ppool.tile([128, NL], f32r, tag=f"qt{h}", name=f"qt{h}") for h in range(2)]
            m_t = [ppool.tile([128, NL], f32r, tag=f"m{h}", name=f"m{h}") for h in range(2)]
            for dst, lhs in ((qT_t, wq_t), (m_t, wkt_t)):
                src = xsT_sb if dst is qT_t else qT_t
                for mh in range(2):
                    for nh in range(2):
                        pp = ps_s.tile([128, 512], f32, tag="scores", name="scores")
                        for kp in range(2):
                            nc.tensor.matmul(
                                pp[:],
                                lhs[kp][:, mh * 128:(mh + 1) * 128],
                                src[kp][:, nh * 512:(nh + 1) * 512],
                                start=(kp == 0), stop=(kp == 1),
                            )
                        nc.vector.tensor_copy(
                            dst[mh][:, nh * 512:(nh + 1) * 512], pp[:])

            # ---- persistent accumulators ----
            ut_ps = [ps_ut.tile([128, NL], f32, tag=f"ut{h}", name=f"ut{h}") for h in range(2)]
            sums_ps = [ps_sum.tile([1, 512], f32, tag=f"sums{h}", name=f"sums{h}")
                       for h in range(2)]

            # ---- main k-loop ----
            for sb in range(N // (KC * SB)):
                xt_t = [xtpool.tile([128, KC * SB], f32r, tag=f"xt{h}", name=f"xt{h}")
                        for h in range(2)]
                for h in range(2):
                    nc.sync.dma_start(
                        xt_t[h][:],
                        xgT[sb * 2 * 128 + h * 128:sb * 2 * 128 + (h + 1) * 128,
                            :])
                xn_t = xnpool.tile([128, SB, D], f32r, tag="xn", name="xn")
                nc.sync.dma_start(
                    xn_t[:],
                    xg[sb * KC * SB:(sb + 1) * KC * SB, :]
                    .rearrange("(a p) d -> p a d", p=128))

                for j in range(SB):
                    c = sb * SB + j
                    first, last = (c == 0), (c == NCHUNK - 1)
                    exps = []
                    for qh in range(2):
                        sp = ps_s.tile([128, 512], f32, tag="scores", name="scores")
                        for kp in range(2):
                            nc.tensor.matmul(
                                sp[:],
                                xt_t[kp][:, j * KC:(j + 1) * KC],
                                m_t[kp][:, qh * 512:(qh + 1) * 512],
                                start=(kp == 0), stop=(kp == 1),
                            )
                        et = epool.tile([128, 512], f32r, tag="expt", name="expt")
                        nc.scalar.activation(et[:], sp[:], EXP, bias=bias_t[:])
                        exps.append(et)
                    for qh in range(2):
                        et = exps[qh]
                        nc.tensor.matmul(
                            sums_ps[qh][:], ones_col[:], et[:],
                            start=first, stop=last)
                        for dh in range(2):
                            nc.tensor.matmul(
                                ut_ps[dh][:, qh * 512:(qh + 1) * 512],
                                xn_t[:, j, dh * 128:(dh + 1) * 128],
                                et[:],
                                start=first, stop=last)

            # ---- tail: softmax normalize + WV projection ----
            sums_sb = tpool.tile([1, NL], f32, tag="sums_sb", name="sums_sb")
            for qh in range(2):
                nc.vector.tensor_copy(
                    sums_sb[:, qh * 512:(qh + 1) * 512], sums_ps[qh][:])
            recip_sb = tpool.tile([1, NL], f32r, tag="recip_sb", name="recip_sb")
            with nc.allow_low_precision(reason="f32r is 4-byte, same mantissa path"):
                nc.vector.reciprocal(recip_sb[:], sums_sb[:])

            rb_sb = tpool.tile([128, NL], f32, tag="rb_sb", name="rb_sb")
            for qh in range(2):
                rp = ps_s.tile([128, 512], f32, tag="scores", name="scores")
                nc.tensor.matmul(
                    rp[:], ones_row[:],
                    recip_sb[:, qh * 512:(qh + 1) * 512],
                    start=True, stop=True)
                nc.vector.tensor_copy(rb_sb[:, qh * 512:(qh + 1) * 512], rp[:])

            utn_sb = [tpool.tile([128, NL], f32r, tag=f"utn{h}", name=f"utn{h}")
                      for h in range(2)]
            for dh in range(2):
                nc.vector.tensor_mul(utn_sb[dh][:], ut_ps[dh][:], rb_sb[:])

            # WV projection into two live PSUM tiles, then int8-quantize with a
            # per-core absmax scale (int8 + f32 scale halves the output bytes;
            # quantization error ~m/240 is ~4e-3 of the rel-err denominator)
            o_f32 = []
            am = tpool.tile([128, 2], f32, tag="am", name="am")
            for mh in range(2):
                op = ps_ut.tile([128, NL], f32, tag=f"ut{mh}", name=f"ut{mh}")
                for nh in range(2):
                    for kp in range(2):
                        nc.tensor.matmul(
                            op[:, nh * 512:(nh + 1) * 512],
                            wv_t[kp][:, mh * 128:(mh + 1) * 128],
                            utn_sb[kp][:, nh * 512:(nh + 1) * 512],
                            start=(kp == 0), stop=(kp == 1),
                        )
                of = tpool.tile([128, NL], f32, tag=f"of{mh}", name=f"of{mh}")
                nc.vector.tensor_copy(of[:], op[:])
                nc.vector.reduce_max(
                    am[:, mh:mh + 1], of[:], axis=mybir.AxisListType.X,
                    apply_absolute_value=True)
                o_f32.append(of)
            amax = tpool.tile([128, 1], f32, tag="amax", name="amax")
            nc.vector.reduce_max(amax[:], am[:], axis=mybir.AxisListType.X)
            nc.gpsimd.partition_all_reduce(
                amax[:], amax[:], channels=128,
                reduce_op=bass_isa.ReduceOp.absmax)
            sc126 = tpool.tile([128, 1], f32, tag="sc126", name="sc126")
            nc.scalar.activation(sc126[:], amax[:], COPY, scale=1.0 / QDEN)
            rcp = tpool.tile([128, 1], f32, tag="rcp", name="rcp")
            nc.vector.reciprocal(rcp[:], sc126[:])
            o_sb = [tpool.tile([128, NL], i8, tag=f"osb{h}", name=f"osb{h}") for h in range(2)]
            for mh in range(2):
                with nc.allow_low_precision(reason="int8 output transport"):
                    nc.vector.tensor_scalar_mul(o_sb[mh][:], o_f32[mh][:],
                                                rcp[:])
                # gpsimd queue, NOT sync: late sync-queue stores corrupt the
                # payload in this build (32-bit words get an fp32-style
                # low-12-bit rounding); the gpsimd DGE ring is clean.
                nc.gpsimd.dma_start(
                    outT[mh * 128:(mh + 1) * 128, :], o_sb[mh][:])
            # absmax f32 bitcast to 4 bytes, packed into outT's extra row
            nc.gpsimd.dma_start(outT[D:D + 1, 0:4],
                                amax[0:1, 0:1].bitcast(i8))

    nc.compile()
    return nc


def _setup_jax_cache():
    """Persistent XLA compilation cache: run_bass_kernel_spmd re-jits a fresh
    closure every call, so without this each call pays ~100ms of XLA
    recompile for the identical HLO."""
    if "jaxcache" in _CACHE:
        return
    import jax

    jax.config.update("jax_compilation_cache_dir", "/tmp/jaxcache")
    jax.config.update("jax_persistent_cache_min_entry_size_bytes", 0)
    jax.config.update("jax_persistent_cache_min_compile_time_secs", 0)
    _CACHE["jaxcache"] = True


def _get_nc():
    if "nc" not in _CACHE:
        _setup_jax_cache()
        _CACHE["nc"] = _build()
    return _CACHE["nc"]


def make_in_maps(input, WQ, WK, WV):
    """Per-core input maps: own fp16 x shard + 1/8 of packed [WQ; WK.T; WV],
    fused into one array (fewer tunnel transfers)."""
    xh = np.ascontiguousarray(input, dtype=np.float32).astype(np.float16)
    wpack = np.concatenate(
        [np.asarray(WQ, dtype=np.float32),
         np.asarray(WK, dtype=np.float32).T,
         np.asarray(WV, dtype=np.float32)], axis=0).astype(np.float16)
    return [{
        "xw_h": np.concatenate(
            [xh[c * NL:(c + 1) * NL], wpack[c * WSH:(c + 1) * WSH]], axis=0),
    } for c in range(P)]


def kernel(input, WQ, WK, WV):
    from concourse import bass_utils

    nc = _get_nc()
    in_maps = make_in_maps(input, WQ, WK, WV)
    res = bass_utils.run_bass_kernel_spmd(nc, in_maps, core_ids=list(range(P)))
    out = np.empty((N, D), dtype=np.float32)
    for c in range(P):
        o = res.results[c]["outT"]
        amax = np.frombuffer(o[D, 0:4].tobytes(), np.float32)[0]
        out[c * NL:(c + 1) * NL, :] = (
            o[:D].astype(np.float32) * (float(amax) / QDEN)).T
    return out
